# revision 16
# baseline (speedup 1.0000x reference)
"""Trainium2 Bass kernel for nn_AttentionStack (4-branch sparse attention).

Sharding: 8 cores = 2 batches x 4 head-pairs (2 heads each). Each core:
  - projects its batch's tokens to Q^T/K^T (ch, tok) and V (tok, ch) for its 2 heads
  - runs the 4 attention branches with scores in (key, query) layout:
      softmax denominators come free via a 2.0-column appended to V
      (the 2.0 also folds the (bb+bo)/2 and (ot+ok)/2 averaging)
  - normalized per-head outputs accumulate into avAll (128 head-dims, 7016 tok)
  - 8-way AllToAll redistributes 1754-token chunks (mirrored across batch
    groups; per-core fc weights are zeroed for cross-batch source chunks)
  - fc over the received (1024, 1754) with 8-step PSUM accumulation + bias
Host only slices/transposes/concats (shard + unshard).
"""
import sys, os

for _p in ("/opt/trn_rl_repo", "/root/.axon_site/_ro/trn_rl_repo"):
    if os.path.isdir(_p) and _p not in sys.path:
        sys.path.insert(0, _p)

import numpy as np
import concourse.bass as bass
import concourse.mybir as mybir
import concourse.tile as tile
from concourse import bacc
from concourse.bass_utils import run_bass_kernel_spmd

F32 = mybir.dt.float32
F32R = mybir.dt.float32r
U8 = mybir.dt.uint8
EXP = mybir.ActivationFunctionType.Exp
MUL = mybir.AluOpType.mult
ADD = mybir.AluOpType.add

NCORES = 8
B = 2
DIM = 512
NH = 8
DK = 64
T1, T2, K, GG = 9, 8, 8, 64
GGP5 = 69
HW = 256
NB = T2 * HW              # 2048 base tokens
FR = K * GGP5             # 552 obj tokens per frame
NO = T1 * FR              # 4968 obj tokens
NT = NB + NO              # 7016
TS = NT // 4              # 1754 tokens per output slice
OT = T1 * GGP5            # 621 tokens per object (ot branch)
OPAD = 640                # per-object padded stride in gathered ot buffers
L_BBOX = T2 * K * 5       # 320
L_GLIM = T1 * K * GG      # 4608

# k-tile partition sizes
def ktiles(n):
    out = []
    while n > 0:
        out.append(min(128, n))
        n -= out[-1]
    return out

KT_BB = ktiles(NB)        # 16 x 128
KT_FR = ktiles(FR)        # [128]*4 + [40]
KT_OT = ktiles(OT)        # [128]*4 + [109]
QB_BB = [(i * 512, 512) for i in range(4)]
QB_OO = [(0, 276), (276, 276)]
QB_OT = [(0, 5, 345, 346), (5, 4, 276, 276)]  # (t0, nt, w_out, w_mm even)
FC_TB = [(0, 440), (440, 440), (880, 440), (1320, 434)]


def build_nc():
    nc = bacc.Bacc("TRN2", target_bir_lowering=False, debug=False,
                   num_devices=NCORES)

    xT = nc.dram_tensor("xT", [DIM, NT], F32R, kind="ExternalInput")
    wq = nc.dram_tensor("wq", [4, 128, 128], F32R, kind="ExternalInput")
    wk = nc.dram_tensor("wk", [4, 128, 128], F32R, kind="ExternalInput")
    wv = nc.dram_tensor("wv", [4, 128, 128], F32R, kind="ExternalInput")
    fcw = nc.dram_tensor("fcw", [8, 128, DIM], F32R, kind="ExternalInput")
    fcb = nc.dram_tensor("fcb", [128, 4], F32, kind="ExternalInput")
    m_bb = nc.dram_tensor("m_bb", [16, 128, NB], U8, kind="ExternalInput")
    m_ot = nc.dram_tensor("m_ot", [5, 128, OT], U8, kind="ExternalInput")
    m_oo = nc.dram_tensor("m_oo", [T1, 5, 128, FR], U8, kind="ExternalInput")
    out = nc.dram_tensor("out", [DIM, TS], F32, kind="ExternalOutput")

    with tile.TileContext(nc) as tc:
        _body(nc, tc, xT, wq, wk, wv, fcw, fcb, m_bb, m_ot, m_oo, out)
    nc.compile()
    return nc


def _body(nc, tc, xT, wq, wk, wv, fcw, fcb, m_bb, m_ot, m_oo, out):
    from concourse.masks import make_identity

    TBLOCKS = []
    t0 = 0
    while t0 < NT:
        w = min(512, NT - t0)
        TBLOCKS.append((t0, w))
        t0 += w

    from contextlib import ExitStack
    with ExitStack() as _stk:
        dram = _stk.enter_context(
            tc.tile_pool(name="dram", bufs=1, space="DRAM"))
        _bigstk = _stk.enter_context(ExitStack())
        big = _bigstk.enter_context(tc.tile_pool(name="big", bufs=1))
        QT = big.tile([128, NT], F32R)      # (head-dims, tok) both heads
        KT = big.tile([128, NT], F32R)
        KTOT = big.tile([128, K * OPAD], F32R)   # ot-gathered keys
        QTOT = big.tile([128, K * OT], F32R)     # ot-gathered queries
        V_BB = big.tile([128, 16 * 130], F32R)   # base V tiles [v_h0|2|v_h1|2]
        V_OO = big.tile([128, T1 * 5 * 130], F32R)  # per-frame-padded obj V
        V_OT = big.tile([128, K * 5 * 130], F32R)   # per-object-padded ot V

        # ---------------- phase 1: projections ----------------
        with (
            tc.tile_pool(name="wpool", bufs=1) as wpool,
            tc.tile_pool(name="vtpool", bufs=1) as vtpool,
            tc.tile_pool(name="xpool", bufs=3) as xpool,
            tc.tile_pool(name="ppsum", bufs=4, space="PSUM") as ppsum,
            tc.tile_pool(name="tpsum", bufs=4, space="PSUM") as tpsum,
        ):
            ident = wpool.tile([128, 128], F32)
            make_identity(nc, ident[:])
            wq_s = wpool.tile([128, 4, 128], F32R)
            wk_s = wpool.tile([128, 4, 128], F32R)
            wv_s = wpool.tile([128, 4, 128], F32R)
            for w_d, w_s in ((wq, wq_s), (wk, wk_s), (wv, wv_s)):
                nc.sync.dma_start(w_s[:], w_d[:].rearrange("t p c -> p t c"))

            # V^T spills to DRAM; transposes stream chunks back (saves SBUF)
            VTD = dram.tile([128, NT], F32)

            for t0, w in TBLOCKS:
                xd = xpool.tile([128, 4, 512], F32R)
                for dt in range(4):
                    nc.sync.dma_start(xd[:, dt, :w],
                                      xT[dt * 128:(dt + 1) * 128, t0:t0 + w])
                for w_s, dst, eng in ((wq_s, QT, nc.vector),
                                      (wk_s, KT, nc.scalar),
                                      (wv_s, None, nc.vector)):
                    ps = ppsum.tile([128, 512], F32)
                    for dt in range(4):
                        nc.tensor.matmul(ps[:, :w], w_s[:, dt, :], xd[:, dt, :w],
                                         start=(dt == 0), stop=(dt == 3))
                    if dst is None:
                        vtmp = xpool.tile([128, 512], F32)
                        nc.vector.tensor_copy(vtmp[:, :w], ps[:, :w])
                        nc.sync.dma_start(VTD[:, t0:t0 + w], vtmp[:, :w])
                    elif eng is nc.scalar:
                        nc.scalar.activation(dst[:, t0:t0 + w], ps[:, :w],
                                             mybir.ActivationFunctionType.Copy)
                    else:
                        eng.tensor_copy(dst[:, t0:t0 + w], ps[:, :w])

            # gathers for the ot branch (free-dim gathers on DVE)
            kt_objview = KT[:, NB:NB + NO].rearrange("c (t z) -> c t z", t=T1)
            qt_objview = QT[:, NB:NB + NO].rearrange("c (t z) -> c t z", t=T1)
            for o in range(K):
                dst = KTOT[:, o * OPAD:o * OPAD + OT].rearrange(
                    "c (t g) -> c t g", t=T1)
                nc.vector.tensor_copy(
                    dst, kt_objview[:, :, o * GGP5:(o + 1) * GGP5])
                dst = QTOT[:, o * OT:(o + 1) * OT].rearrange(
                    "c (t g) -> c t g", t=T1)
                nc.vector.tensor_copy(
                    dst, qt_objview[:, :, o * GGP5:(o + 1) * GGP5])

            # transposes: VT (ch, tok) -> V natural (tok, ch) tiles with the
            # 2.0 denominator columns at 64 and 129
            def build_v(v_dst, tile_idx, src_ap, w):
                for h in range(2):
                    tp = tpsum.tile([128, 64], F32)
                    nc.tensor.transpose(tp[:w, :], src_ap[h * 64:h * 64 + 64, :],
                                        ident[h * 64:h * 64 + 64,
                                              h * 64:h * 64 + 64])
                    nc.vector.tensor_copy(
                        v_dst[:w, tile_idx * 130 + 65 * h:tile_idx * 130 + 65 * h + 64],
                        tp[:w, :])

            for tt in range(0, 16, 4):
                ch = xpool.tile([128, 512], F32)
                nc.sync.dma_start(ch[:], VTD[:, tt * 128:tt * 128 + 512])
                for j in range(4):
                    build_v(V_BB, tt + j, ch[:, j * 128:(j + 1) * 128], 128)
            for f in range(T1):
                ch = xpool.tile([128, 552], F32, tag="chf")
                nc.sync.dma_start(ch[:], VTD[:, NB + f * FR:NB + (f + 1) * FR])
                for kt, kw in enumerate(KT_FR):
                    build_v(V_OO, f * 5 + kt, ch[:, kt * 128:kt * 128 + kw], kw)
            vtd_obj = VTD[:, NB:NB + NO].rearrange("c (t z) -> c t z", t=T1)
            for o in range(K):
                ch = xpool.tile([128, OT], F32, tag="cho")
                nc.sync.dma_start(
                    ch[:].rearrange("c (t g) -> c t g", t=T1),
                    vtd_obj[:, :, o * GGP5:(o + 1) * GGP5])
                for kt, kw in enumerate(KT_OT):
                    build_v(V_OT, o * 5 + kt, ch[:, kt * 128:kt * 128 + kw], kw)

            for v_dst, n in ((V_BB, 16), (V_OO, T1 * 5), (V_OT, K * 5)):
                vv = v_dst[:].bitcast(F32).rearrange("p (n c) -> p n c", c=130)
                nc.gpsimd.memset(vv[:, :, 64:65], 2.0)
                nc.gpsimd.memset(vv[:, :, 129:130], 2.0)

        # ---------------- phase 2: attention ----------------
        with (
            tc.tile_pool(name="avall", bufs=1) as avallp,
            tc.tile_pool(name="motp", bufs=1) as motp,
            tc.tile_pool(name="stps", bufs=3, space="PSUM") as stps,
            tc.tile_pool(name="avps", bufs=2, space="PSUM") as avpsp,
            tc.tile_pool(name="ptp", bufs=3) as ptp,
            tc.tile_pool(name="pmp", bufs=3) as pmp,
            tc.tile_pool(name="mtp", bufs=3) as mtp,
            tc.tile_pool(name="nrm", bufs=2) as nrm,
        ):
            avAll = avallp.tile([128, NT], F32R)
            mot_s = motp.tile([128, 5, OT], U8)
            nc.sync.dma_start(mot_s[:], m_ot[:].rearrange("t p c -> p t c"))

            def branch(hs, q_ap, q0g, wq_, lhsT_fn, v_fn, ksizes, mask_fn,
                       first_write):
                """One (head, branch, q-block) attention pass.
                q_ap: rhs AP (64, ...) covering wq_ query columns
                q0g: global token offset of this q-block in avAll
                lhsT_fn(kt): (64, kw) key AP;  v_fn(kt): (kw, 65) V' AP
                mask_fn(kt, kw): mask AP (kw, wq_) or None
                """
                nkt = len(ksizes)
                avp = avpsp.tile([65, 512], F32)
                for kt, kw in enumerate(ksizes):
                    stp = stps.tile([128, 512], F32)
                    nc.tensor.matmul(stp[:kw, :wq_], lhsT_fn(kt), q_ap,
                                     start=True, stop=True)
                    pt = ptp.tile([128, 512], F32R)
                    nc.scalar.activation(pt[:kw, :wq_], stp[:kw, :wq_], EXP,
                                         scale=0.125)
                    rhs = pt
                    m_ap = mask_fn(kt, kw)
                    if m_ap is not None:
                        mt = mtp.tile([128, 512], U8)
                        nc.sync.dma_start(mt[:kw, :wq_], m_ap)
                        pm = pmp.tile([128, 512], F32R)
                        nc.vector.tensor_tensor(pm[:kw, :wq_], pt[:kw, :wq_],
                                                mt[:kw, :wq_], op=MUL)
                        rhs = pm
                    nc.tensor.matmul(avp[:, :wq_], v_fn(kt), rhs[:kw, :wq_],
                                     start=(kt == 0), stop=(kt == nkt - 1))
                rc = nrm.tile([1, 512], F32)
                nc.vector.reciprocal(rc[:, :wq_], avp[64:65, :wq_])
                bc = nrm.tile([64, 512], F32)
                nc.gpsimd.partition_broadcast(bc[:, :wq_], rc[:, :wq_])
                tmp = nrm.tile([64, 512], F32R)
                nc.vector.tensor_tensor(tmp[:, :wq_], avp[0:64, :wq_],
                                        bc[:, :wq_], op=MUL)
                # move into avAll via DMA: no base-partition constraint,
                # and accum_op=add folds the second branch of each pair
                nc.gpsimd.dma_start(avAll[hs:hs + 64, q0g:q0g + wq_],
                                    tmp[:, :wq_],
                                    accum_op=(mybir.AluOpType.bypass
                                              if first_write else ADD))

            for h in range(2):
                hs = h * 64

                # base <- base (causal mask)
                for q0, wq_ in QB_BB:
                    branch(
                        hs, QT[hs:hs + 64, q0:q0 + wq_], q0, wq_,
                        lambda kt: KT[hs:hs + 64, kt * 128:(kt + 1) * 128],
                        lambda kt: V_BB[:, kt * 130 + 65 * h:kt * 130 + 65 * h + 65],
                        KT_BB,
                        lambda kt, kw: m_bb[kt, 0:kw, q0:q0 + wq_],
                        first_write=True)

                # base <- objects (per frame, no mask)
                for f in range(T2):
                    q0 = f * HW
                    branch(
                        hs, QT[hs:hs + 64, q0:q0 + HW], q0, HW,
                        lambda kt, f=f: KT[hs:hs + 64,
                                           NB + f * FR + kt * 128:
                                           NB + f * FR + kt * 128 + KT_FR[kt]],
                        lambda kt, f=f: V_OO[0:KT_FR[kt],
                                             (f * 5 + kt) * 130 + 65 * h:
                                             (f * 5 + kt) * 130 + 65 * h + 65],
                        KT_FR,
                        lambda kt, kw: None,
                        first_write=False)

                # objects <- objects per frame (oo mask)
                for f in range(T1):
                    for q0l, wq_ in QB_OO:
                        qg = NB + f * FR + q0l
                        branch(
                            hs, QT[hs:hs + 64, qg:qg + wq_], qg, wq_,
                            lambda kt, f=f: KT[hs:hs + 64,
                                               NB + f * FR + kt * 128:
                                               NB + f * FR + kt * 128 + KT_FR[kt]],
                            lambda kt, f=f: V_OO[0:KT_FR[kt],
                                                 (f * 5 + kt) * 130 + 65 * h:
                                                 (f * 5 + kt) * 130 + 65 * h + 65],
                            KT_FR,
                            lambda kt, kw, f=f, q0l=q0l, wq_=wq_:
                                m_oo[f, kt, 0:kw, q0l:q0l + wq_],
                            first_write=True)

                # object over time, per object (ot mask); scatter-add output
                av_obj = avAll[hs:hs + 64, NB:NB + NO].rearrange(
                    "c (t z) -> c t z", t=T1)
                for o in range(K):
                    for t0_, nt_, wq_, wm_ in QB_OT:
                        qoff = t0_ * GGP5
                        q_ap = QTOT[hs:hs + 64,
                                    o * OT + qoff:o * OT + qoff + wm_]
                        avp = avpsp.tile([65, 512], F32)
                        for kt, kw in enumerate(KT_OT):
                            stp = stps.tile([128, 512], F32)
                            nc.tensor.matmul(
                                stp[:kw, :wm_],
                                KTOT[hs:hs + 64,
                                     o * OPAD + kt * 128:o * OPAD + kt * 128 + kw],
                                q_ap, start=True, stop=True)
                            pt = ptp.tile([128, 512], F32R)
                            nc.scalar.activation(pt[:kw, :wm_], stp[:kw, :wm_],
                                                 EXP, scale=0.125)
                            mt = mtp.tile([128, 512], U8)
                            nc.sync.dma_start(mt[:kw, :wq_],
                                              m_ot[kt, 0:kw, qoff:qoff + wq_])
                            if wm_ > wq_:
                                nc.gpsimd.memset(mt[:kw, wq_:wm_], 0)
                            pm = pmp.tile([128, 512], F32R)
                            nc.vector.tensor_tensor(pm[:kw, :wm_], pt[:kw, :wm_],
                                                    mt[:kw, :wm_], op=MUL)
                            nc.tensor.matmul(
                                avp[:, :wm_],
                                V_OT[0:kw, (o * 5 + kt) * 130 + 65 * h:
                                     (o * 5 + kt) * 130 + 65 * h + 65],
                                pm[:kw, :wm_],
                                start=(kt == 0), stop=(kt == len(KT_OT) - 1))
                        rc = nrm.tile([1, 512], F32)
                        nc.vector.reciprocal(rc[:, :wq_], avp[64:65, :wq_])
                        bc = nrm.tile([64, 512], F32)
                        nc.gpsimd.partition_broadcast(bc[:, :wq_], rc[:, :wq_])
                        tmp = nrm.tile([64, 512], F32R)
                        nc.vector.tensor_tensor(tmp[:, :wq_], avp[0:64, :wq_],
                                                bc[:, :wq_], op=MUL)
                        dst = av_obj[:, t0_:t0_ + nt_, o * GGP5:(o + 1) * GGP5]
                        src = tmp[:, :wq_].rearrange("c (t g) -> c t g", t=nt_)
                        nc.gpsimd.dma_start(dst, src, accum_op=ADD)

            # ---------------- phase 3: all-to-all ----------------
            a2a_in = dram.tile([NCORES, 128, TS], F32R)
            a2a_out = dram.tile([NCORES, 128, TS], F32R)
            for d in range(NCORES):
                nc.sync.dma_start(a2a_in[d],
                                  avAll[:, (d % 4) * TS:(d % 4) * TS + TS])
            nc.gpsimd.collective_compute(
                "AllToAll", mybir.AluOpType.bypass,
                replica_groups=[list(range(NCORES))],
                ins=[a2a_in[:].opt()], outs=[a2a_out[:].opt()])

        # ---------------- phase 4: fc ----------------
        _bigstk.close()
        with (
            tc.tile_pool(name="fcpool", bufs=1) as fcpool,
            tc.tile_pool(name="fops", bufs=4, space="PSUM") as fops,
            tc.tile_pool(name="fout", bufs=3) as fout,
        ):
            av_rx = fcpool.tile([128, NCORES, TS], F32R)
            fcw_s = fcpool.tile([128, NCORES, DIM], F32R)
            fcb_s = fcpool.tile([128, 4], F32)
            nc.sync.dma_start(fcw_s[:], fcw[:].rearrange("s p o -> p s o"))
            nc.sync.dma_start(fcb_s[:], fcb[:])
            for s in range(NCORES):
                nc.sync.dma_start(av_rx[:, s, :], a2a_out[s])

            for o4 in range(4):
                for tb0, tbw in FC_TB:
                    fps = fops.tile([128, 440], F32)
                    for s in range(NCORES):
                        nc.tensor.matmul(
                            fps[:, :tbw],
                            fcw_s[:, s, o4 * 128:(o4 + 1) * 128],
                            av_rx[:, s, tb0:tb0 + tbw],
                            start=(s == 0), stop=(s == NCORES - 1))
                    ot_t = fout.tile([128, 440], F32)
                    nc.vector.tensor_scalar(ot_t[:, :tbw], fps[:, :tbw],
                                            fcb_s[:, o4:o4 + 1], None, op0=ADD)
                    nc.sync.dma_start(out[o4 * 128:(o4 + 1) * 128,
                                          tb0:tb0 + tbw], ot_t[:, :tbw])


_NC_CACHE = None


def _get_nc():
    global _NC_CACHE
    if _NC_CACHE is None:
        _NC_CACHE = build_nc()
    return _NC_CACHE


def _prep_core_inputs(x, wq, wk, wv, fc_w, fc_b, base_mask, ot_mask, oo_mask):
    """Build the 8 per-core input maps (host-side shard/pack)."""
    x = np.asarray(x, np.float32)
    maps = []

    # per-batch packed token stream: [base(2048) | obj(4968)] transposed
    xT_b = []
    for b in range(B):
        gl = x[b, L_BBOX + NB:L_BBOX + NB + L_GLIM].reshape(T1, K, GG, DIM)
        bb = x[b, :L_BBOX].reshape(T2, K, 5, DIM)
        bbp = np.zeros((T1, K, 5, DIM), np.float32)
        bbp[:T2] = bb
        x_obj = np.concatenate([gl, bbp], axis=2).reshape(NO, DIM)
        x_base = x[b, L_BBOX:L_BBOX + NB]
        xa = np.concatenate([x_base, x_obj], axis=0)          # (7016, 512)
        xT_b.append(np.ascontiguousarray(xa.T))               # (512, 7016)

    def pack_w(w, cols):
        # (512, 128) head-pair columns -> (4, 128, 128) d-major tiles
        return np.ascontiguousarray(
            np.asarray(w, np.float32)[:, cols].reshape(4, 128, 128))

    # masks (shared across cores): transposed to (k, q), padded, tiled
    def tile_mask(mT, nk):                                    # mT (k, q)
        kp = len(ktiles(nk)) * 128
        mp = np.zeros((kp, mT.shape[1]), np.uint8)
        mp[:nk] = mT.astype(np.uint8)
        return np.ascontiguousarray(mp.reshape(-1, 128, mT.shape[1]))

    mbb = tile_mask(np.asarray(base_mask).T, NB)              # (16,128,2048)
    mot = tile_mask(np.asarray(ot_mask).T, OT)                # (5,128,621)
    moo = np.stack([tile_mask(np.asarray(oo_mask[f]).T, FR) for f in range(T1)])

    fcb_t = np.ascontiguousarray(
        np.asarray(fc_b, np.float32).reshape(4, 128).T)       # (128, 4)

    fc_w = np.asarray(fc_w, np.float32)
    for c in range(NCORES):
        b, hp = c // 4, c % 4
        cols = slice(hp * 128, hp * 128 + 128)
        fcw_full = np.zeros((NCORES, 128, DIM), np.float32)
        grp = range(4) if b == 0 else range(4, 8)
        for s in grp:
            fcw_full[s] = fc_w[(s % 4) * 128:(s % 4) * 128 + 128, :]
        maps.append({
            "xT": xT_b[b],
            "wq": pack_w(wq, cols), "wk": pack_w(wk, cols),
            "wv": pack_w(wv, cols),
            "fcw": fcw_full, "fcb": fcb_t,
            "m_bb": mbb, "m_ot": mot, "m_oo": moo,
        })
    return maps


def _assemble(results):
    p_full = np.empty((B, NT, DIM), np.float32)
    for c in range(NCORES):
        b, sl = c // 4, c % 4
        p_full[b, sl * TS:(sl + 1) * TS] = results[c]["out"].T
    p_base = p_full[:, :NB].reshape(B, T2, 16, 16, DIM)
    obj = p_full[:, NB:].reshape(B, T1, K, GGP5, DIM)
    p_glimpses = np.ascontiguousarray(
        obj[..., :GG, :].reshape(B, T1, K, 8, 8, DIM))
    p_bboxes = np.ascontiguousarray(obj[:, :T2, :, GG:, :])
    return (p_bboxes, np.ascontiguousarray(p_base), p_glimpses)


def _ensure_ntff_hook():
    """Register the axon NTFF profile hook if the image's antenv lacks it."""
    import types
    try:
        from antenv.axon_hooks import get_axon_ntff_profile_hook  # noqa: F401
        return
    except ImportError:
        pass
    try:
        import antenv
        mod = types.ModuleType("antenv.axon_hooks")
        state = {"hook": None}
        mod.set_axon_ntff_profile_hook = lambda h: state.__setitem__("hook", h)
        mod.get_axon_ntff_profile_hook = lambda: state["hook"]
        sys.modules["antenv.axon_hooks"] = mod
        antenv.axon_hooks = mod
        if "/root/.axon_site" not in sys.path:
            sys.path.insert(0, "/root/.axon_site")
        from trn_agent_boot.trn_boot import _ntff_profile_via_ctypes
        hook = _ntff_profile_via_ctypes("/opt/axon/libaxon_pjrt.so")
        if hook is not None:
            mod.set_axon_ntff_profile_hook(hook)
    except Exception:
        pass


def run(trace=False, **inputs):
    if trace:
        _ensure_ntff_hook()
    nc = _get_nc()
    maps = _prep_core_inputs(**inputs)
    res = run_bass_kernel_spmd(nc, maps, list(range(NCORES)), trace=trace)
    return _assemble(res.results), res


def kernel(**inputs):
    out, _ = run(trace=False, **inputs)
    return out


# revision 19
# speedup vs baseline: 1.4073x; 1.4073x over previous
"""Trainium2 Bass kernel for nn_AttentionStack (4-branch sparse attention).

Sharding: 8 cores = 2 batches x 4 head-pairs (2 heads each). Each core:
  - projects its batch's tokens to Q^T/K^T (ch, tok) and V (tok, ch) for its 2 heads
  - runs the 4 attention branches with scores in (key, query) layout:
      softmax denominators come free via a 2.0-column appended to V
      (the 2.0 also folds the (bb+bo)/2 and (ot+ok)/2 averaging)
  - normalized per-head outputs accumulate into avAll (128 head-dims, 7016 tok)
  - 8-way AllToAll redistributes 1754-token chunks (mirrored across batch
    groups; per-core fc weights are zeroed for cross-batch source chunks)
  - fc over the received (1024, 1754) with 8-step PSUM accumulation + bias
Host only slices/transposes/concats (shard + unshard).
"""
import sys, os

for _p in ("/opt/trn_rl_repo", "/root/.axon_site/_ro/trn_rl_repo"):
    if os.path.isdir(_p) and _p not in sys.path:
        sys.path.insert(0, _p)

import numpy as np
import concourse.bass as bass
import concourse.mybir as mybir
import concourse.tile as tile
from concourse import bacc
from concourse.bass_utils import run_bass_kernel_spmd

F32 = mybir.dt.float32
F32R = mybir.dt.float32r
F16 = mybir.dt.float16
U8 = mybir.dt.uint8
EXP = mybir.ActivationFunctionType.Exp
MUL = mybir.AluOpType.mult
ADD = mybir.AluOpType.add

NCORES = 8
B = 2
DIM = 512
NH = 8
DK = 64
T1, T2, K, GG = 9, 8, 8, 64
GGP5 = 69
HW = 256
NB = T2 * HW              # 2048 base tokens
FR = K * GGP5             # 552 obj tokens per frame
NO = T1 * FR              # 4968 obj tokens
NT = NB + NO              # 7016
TS = NT // 4              # 1754 tokens per output slice
OT = T1 * GGP5            # 621 tokens per object (ot branch)
OPAD = 640                # per-object padded stride in gathered ot buffers
L_BBOX = T2 * K * 5       # 320
L_GLIM = T1 * K * GG      # 4608

# k-tile partition sizes
def ktiles(n):
    out = []
    while n > 0:
        out.append(min(128, n))
        n -= out[-1]
    return out

KT_BB = ktiles(NB)        # 16 x 128
KT_FR = ktiles(FR)        # [128]*4 + [40]
KT_OT = ktiles(OT)        # [128]*4 + [109]
QB_BB = [(i * 512, 512) for i in range(4)]
QB_OO = [(0, 276), (276, 276)]
QB_OT = [(0, 5, 345, 346), (5, 4, 276, 276)]  # (t0, nt, w_out, w_mm even)
FC_TB = [(0, 440), (440, 440), (880, 440), (1320, 434)]


def build_nc():
    nc = bacc.Bacc("TRN2", target_bir_lowering=False, debug=False,
                   num_devices=NCORES)

    xT = nc.dram_tensor("xT", [DIM, NT], F16, kind="ExternalInput")
    wq = nc.dram_tensor("wq", [4, 128, 128], F16, kind="ExternalInput")
    wk = nc.dram_tensor("wk", [4, 128, 128], F16, kind="ExternalInput")
    wv = nc.dram_tensor("wv", [4, 128, 128], F16, kind="ExternalInput")
    fcw = nc.dram_tensor("fcw", [8, 128, DIM], F16, kind="ExternalInput")
    fcb = nc.dram_tensor("fcb", [128, 4], F32, kind="ExternalInput")
    m_bb = nc.dram_tensor("m_bb", [16, 128, NB], U8, kind="ExternalInput")
    m_ot = nc.dram_tensor("m_ot", [5, 128, OT], U8, kind="ExternalInput")
    m_oo = nc.dram_tensor("m_oo", [T1, 5, 128, FR], U8, kind="ExternalInput")
    out = nc.dram_tensor("out", [DIM, TS], F32, kind="ExternalOutput")

    with tile.TileContext(nc) as tc:
        _body(nc, tc, xT, wq, wk, wv, fcw, fcb, m_bb, m_ot, m_oo, out)
    nc.compile()
    return nc


def _body(nc, tc, xT, wq, wk, wv, fcw, fcb, m_bb, m_ot, m_oo, out):
    from concourse.masks import make_identity

    TBLOCKS = []
    t0 = 0
    while t0 < NT:
        w = min(512, NT - t0)
        TBLOCKS.append((t0, w))
        t0 += w

    from contextlib import ExitStack
    with ExitStack() as _stk:
        dram = _stk.enter_context(
            tc.tile_pool(name="dram", bufs=1, space="DRAM"))
        _bigstk = _stk.enter_context(ExitStack())
        big = _bigstk.enter_context(tc.tile_pool(name="big", bufs=1))
        QT = big.tile([128, NT], F16)      # (head-dims, tok) both heads
        KT = big.tile([128, NT], F16)
        KTOT = big.tile([128, K * OPAD], F16)   # ot-gathered keys
        QTOT = big.tile([128, K * OT], F16)     # ot-gathered queries
        V_BB = big.tile([128, 16 * 130], F16)   # base V tiles [v_h0|2|v_h1|2]
        V_OO = big.tile([128, T1 * 5 * 130], F16)  # per-frame-padded obj V
        V_OT = big.tile([128, K * 5 * 130], F16)   # per-object-padded ot V

        # ---------------- phase 1: projections ----------------
        with (
            tc.tile_pool(name="wpool", bufs=1) as wpool,
            tc.tile_pool(name="vtpool", bufs=1) as vtpool,
            tc.tile_pool(name="xpool", bufs=3) as xpool,
            tc.tile_pool(name="ppsum", bufs=4, space="PSUM") as ppsum,
            tc.tile_pool(name="tpsum", bufs=4, space="PSUM") as tpsum,
        ):
            ident = wpool.tile([128, 128], F16)
            make_identity(nc, ident[:])
            wq_s = wpool.tile([128, 4, 128], F16)
            wk_s = wpool.tile([128, 4, 128], F16)
            wv_s = wpool.tile([128, 4, 128], F16)
            for w_d, w_s in ((wq, wq_s), (wk, wk_s), (wv, wv_s)):
                nc.sync.dma_start(w_s[:], w_d[:].rearrange("t p c -> p t c"))

            # V^T spills to DRAM; transposes stream chunks back (saves SBUF)
            VTD = dram.tile([128, NT], F16)

            for t0, w in TBLOCKS:
                xd = xpool.tile([128, 4, 512], F16)
                nc.sync.dma_start(
                    xd[:, :, :w],
                    xT[:].rearrange("(dt p) t -> p dt t", dt=4)[:, :, t0:t0 + w])
                for w_s, dst, eng in ((wq_s, QT, nc.vector),
                                      (wk_s, KT, nc.scalar),
                                      (wv_s, None, nc.vector)):
                    ps = ppsum.tile([128, 512], F32)
                    for dt in range(4):
                        nc.tensor.matmul(ps[:, :w], w_s[:, dt, :], xd[:, dt, :w],
                                         start=(dt == 0), stop=(dt == 3))
                    if dst is None:
                        vtmp = xpool.tile([128, 512], F16)
                        nc.vector.tensor_copy(vtmp[:, :w], ps[:, :w])
                        nc.sync.dma_start(VTD[:, t0:t0 + w], vtmp[:, :w])
                    elif eng is nc.scalar:
                        nc.scalar.activation(dst[:, t0:t0 + w], ps[:, :w],
                                             mybir.ActivationFunctionType.Copy)
                    else:
                        eng.tensor_copy(dst[:, t0:t0 + w], ps[:, :w])

            # gathers for the ot branch (free-dim gathers on DVE)
            kt_objview = KT[:, NB:NB + NO].rearrange("c (t z) -> c t z", t=T1)
            qt_objview = QT[:, NB:NB + NO].rearrange("c (t z) -> c t z", t=T1)
            for o in range(K):
                dst = KTOT[:, o * OPAD:o * OPAD + OT].rearrange(
                    "c (t g) -> c t g", t=T1)
                nc.vector.tensor_copy(
                    dst, kt_objview[:, :, o * GGP5:(o + 1) * GGP5])
                dst = QTOT[:, o * OT:(o + 1) * OT].rearrange(
                    "c (t g) -> c t g", t=T1)
                nc.vector.tensor_copy(
                    dst, qt_objview[:, :, o * GGP5:(o + 1) * GGP5])

            # transposes: VT (ch, tok) -> V natural (tok, ch) tiles with the
            # 2.0 denominator columns at 64 and 129
            def build_v(v_dst, tile_idx, src_ap, w):
                for h in range(2):
                    tp = tpsum.tile([128, 64], F16)
                    nc.tensor.transpose(tp[:w, :], src_ap[h * 64:h * 64 + 64, :],
                                        ident[h * 64:h * 64 + 64,
                                              h * 64:h * 64 + 64])
                    nc.vector.tensor_copy(
                        v_dst[:w, tile_idx * 130 + 65 * h:tile_idx * 130 + 65 * h + 64],
                        tp[:w, :])

            for tt in range(0, 16, 4):
                ch = xpool.tile([128, 512], F16)
                nc.sync.dma_start(ch[:], VTD[:, tt * 128:tt * 128 + 512])
                for j in range(4):
                    build_v(V_BB, tt + j, ch[:, j * 128:(j + 1) * 128], 128)
            for f in range(T1):
                ch = xpool.tile([128, 552], F16, tag="chf")
                nc.sync.dma_start(ch[:], VTD[:, NB + f * FR:NB + (f + 1) * FR])
                for kt, kw in enumerate(KT_FR):
                    build_v(V_OO, f * 5 + kt, ch[:, kt * 128:kt * 128 + kw], kw)
            vtd_obj = VTD[:, NB:NB + NO].rearrange("c (t z) -> c t z", t=T1)
            for o in range(K):
                ch = xpool.tile([128, OT], F16, tag="cho")
                nc.sync.dma_start(
                    ch[:].rearrange("c (t g) -> c t g", t=T1),
                    vtd_obj[:, :, o * GGP5:(o + 1) * GGP5])
                for kt, kw in enumerate(KT_OT):
                    build_v(V_OT, o * 5 + kt, ch[:, kt * 128:kt * 128 + kw], kw)

            for v_dst, n in ((V_BB, 16), (V_OO, T1 * 5), (V_OT, K * 5)):
                vv = v_dst[:].rearrange("p (n c) -> p n c", c=130)
                nc.gpsimd.memset(vv[:, :, 64:65], 2.0)
                nc.gpsimd.memset(vv[:, :, 129:130], 2.0)

        # ---------------- phase 2: attention ----------------
        with (
            tc.tile_pool(name="avall", bufs=1) as avallp,
            tc.tile_pool(name="motp", bufs=1) as motp,
            tc.tile_pool(name="stps", bufs=4, space="PSUM") as stps,
            tc.tile_pool(name="avps", bufs=3, space="PSUM") as avpsp,
            tc.tile_pool(name="ptp", bufs=4) as ptp,
            tc.tile_pool(name="pmp", bufs=4) as pmp,
            tc.tile_pool(name="nrm", bufs=3) as nrm,
        ):
            avAll = avallp.tile([128, NT], F16)
            mot_s = motp.tile([128, 5, OT], U8)
            nc.sync.dma_start(mot_s[:], m_ot[:].rearrange("t p c -> p t c"))
            mbb_s = motp.tile([128, 16, NB], U8)
            nc.sync.dma_start(mbb_s[:], m_bb[:].rearrange("t p c -> p t c"))
            moo_s = motp.tile([128, T1, 5, FR], U8)
            nc.sync.dma_start(moo_s[:],
                              m_oo[:].rearrange("f t p c -> p f t c"))

            def branch(hs, q_ap, q0g, wq_, lhsT_fn, v_fn, ksizes, mask_fn,
                       first_write):
                """One (head, branch, q-block) attention pass.
                q_ap: rhs AP (64, ...) covering wq_ query columns
                q0g: global token offset of this q-block in avAll
                lhsT_fn(kt): (64, kw) key AP;  v_fn(kt): (kw, 65) V' AP
                mask_fn(kt, kw): mask AP (kw, wq_) or None
                """
                nkt = len(ksizes)
                avp = avpsp.tile([65, 512], F32)
                for kt, kw in enumerate(ksizes):
                    stp = stps.tile([128, 512], F32)
                    nc.tensor.matmul(stp[:kw, :wq_], lhsT_fn(kt), q_ap,
                                     start=True, stop=True)
                    pt = ptp.tile([128, 512], F16)
                    nc.scalar.activation(pt[:kw, :wq_], stp[:kw, :wq_], EXP,
                                         scale=0.125)
                    rhs = pt
                    m_ap = mask_fn(kt, kw)
                    if m_ap is not None:
                        pm = pmp.tile([128, 512], F16)
                        nc.vector.tensor_tensor(pm[:kw, :wq_], pt[:kw, :wq_],
                                                m_ap, op=MUL)
                        rhs = pm
                    nc.tensor.matmul(avp[:, :wq_], v_fn(kt), rhs[:kw, :wq_],
                                     start=(kt == 0), stop=(kt == nkt - 1))
                rc = nrm.tile([1, 512], F32)
                nc.vector.reciprocal(rc[:, :wq_], avp[64:65, :wq_])
                bc = nrm.tile([64, 512], F32)
                nc.gpsimd.partition_broadcast(bc[:, :wq_], rc[:, :wq_])
                tmp = nrm.tile([64, 512], F16)
                nc.vector.tensor_tensor(tmp[:, :wq_], avp[0:64, :wq_],
                                        bc[:, :wq_], op=MUL)
                # move into avAll via DMA: no base-partition constraint,
                # and accum_op=add folds the second branch of each pair
                nc.gpsimd.dma_start(avAll[hs:hs + 64, q0g:q0g + wq_],
                                    tmp[:, :wq_],
                                    accum_op=(mybir.AluOpType.bypass
                                              if first_write else ADD))

            for h in range(2):
                hs = h * 64

                # base <- base (causal mask)
                for q0, wq_ in QB_BB:
                    branch(
                        hs, QT[hs:hs + 64, q0:q0 + wq_], q0, wq_,
                        lambda kt: KT[hs:hs + 64, kt * 128:(kt + 1) * 128],
                        lambda kt: V_BB[:, kt * 130 + 65 * h:kt * 130 + 65 * h + 65],
                        KT_BB,
                        lambda kt, kw: mbb_s[0:kw, kt, q0:q0 + wq_],
                        first_write=True)

                # base <- objects (per frame, no mask)
                for f in range(T2):
                    q0 = f * HW
                    branch(
                        hs, QT[hs:hs + 64, q0:q0 + HW], q0, HW,
                        lambda kt, f=f: KT[hs:hs + 64,
                                           NB + f * FR + kt * 128:
                                           NB + f * FR + kt * 128 + KT_FR[kt]],
                        lambda kt, f=f: V_OO[0:KT_FR[kt],
                                             (f * 5 + kt) * 130 + 65 * h:
                                             (f * 5 + kt) * 130 + 65 * h + 65],
                        KT_FR,
                        lambda kt, kw: None,
                        first_write=False)

                # objects <- objects per frame (oo mask)
                for f in range(T1):
                    for q0l, wq_ in QB_OO:
                        qg = NB + f * FR + q0l
                        branch(
                            hs, QT[hs:hs + 64, qg:qg + wq_], qg, wq_,
                            lambda kt, f=f: KT[hs:hs + 64,
                                               NB + f * FR + kt * 128:
                                               NB + f * FR + kt * 128 + KT_FR[kt]],
                            lambda kt, f=f: V_OO[0:KT_FR[kt],
                                                 (f * 5 + kt) * 130 + 65 * h:
                                                 (f * 5 + kt) * 130 + 65 * h + 65],
                            KT_FR,
                            lambda kt, kw, f=f, q0l=q0l, wq_=wq_:
                                moo_s[0:kw, f, kt, q0l:q0l + wq_],
                            first_write=True)

                # object over time, per object (ot mask); scatter-add output
                av_obj = avAll[hs:hs + 64, NB:NB + NO].rearrange(
                    "c (t z) -> c t z", t=T1)
                for o in range(K):
                    for t0_, nt_, wq_, wm_ in QB_OT:
                        qoff = t0_ * GGP5
                        q_ap = QTOT[hs:hs + 64,
                                    o * OT + qoff:o * OT + qoff + wm_]
                        avp = avpsp.tile([65, 512], F32)
                        for kt, kw in enumerate(KT_OT):
                            stp = stps.tile([128, 512], F32)
                            nc.tensor.matmul(
                                stp[:kw, :wm_],
                                KTOT[hs:hs + 64,
                                     o * OPAD + kt * 128:o * OPAD + kt * 128 + kw],
                                q_ap, start=True, stop=True)
                            pt = ptp.tile([128, 512], F16)
                            nc.scalar.activation(pt[:kw, :wm_], stp[:kw, :wm_],
                                                 EXP, scale=0.125)
                            pm = pmp.tile([128, 512], F16)
                            nc.vector.tensor_tensor(
                                pm[:kw, :wm_], pt[:kw, :wm_],
                                mot_s[0:kw, kt, qoff:qoff + wm_], op=MUL)
                            nc.tensor.matmul(
                                avp[:, :wm_],
                                V_OT[0:kw, (o * 5 + kt) * 130 + 65 * h:
                                     (o * 5 + kt) * 130 + 65 * h + 65],
                                pm[:kw, :wm_],
                                start=(kt == 0), stop=(kt == len(KT_OT) - 1))
                        rc = nrm.tile([1, 512], F32)
                        nc.vector.reciprocal(rc[:, :wq_], avp[64:65, :wq_])
                        bc = nrm.tile([64, 512], F32)
                        nc.gpsimd.partition_broadcast(bc[:, :wq_], rc[:, :wq_])
                        tmp = nrm.tile([64, 512], F16)
                        nc.vector.tensor_tensor(tmp[:, :wq_], avp[0:64, :wq_],
                                                bc[:, :wq_], op=MUL)
                        dst = av_obj[:, t0_:t0_ + nt_, o * GGP5:(o + 1) * GGP5]
                        src = tmp[:, :wq_].rearrange("c (t g) -> c t g", t=nt_)
                        nc.gpsimd.dma_start(dst, src, accum_op=ADD)

            # ---------------- phase 3: all-to-all ----------------
            a2a_in = dram.tile([NCORES, 128, TS], F16)
            a2a_out = dram.tile([NCORES, 128, TS], F16)
            for d in range(NCORES):
                nc.sync.dma_start(a2a_in[d],
                                  avAll[:, (d % 4) * TS:(d % 4) * TS + TS])
            nc.gpsimd.collective_compute(
                "AllToAll", mybir.AluOpType.bypass,
                replica_groups=[list(range(NCORES))],
                ins=[a2a_in[:].opt()], outs=[a2a_out[:].opt()])

        # ---------------- phase 4: fc ----------------
        _bigstk.close()
        with (
            tc.tile_pool(name="fcpool", bufs=1) as fcpool,
            tc.tile_pool(name="fops", bufs=4, space="PSUM") as fops,
            tc.tile_pool(name="fout", bufs=3) as fout,
        ):
            av_rx = fcpool.tile([128, NCORES, TS], F16)
            fcw_s = fcpool.tile([128, NCORES, DIM], F16)
            fcb_s = fcpool.tile([128, 4], F32)
            nc.sync.dma_start(fcw_s[:], fcw[:].rearrange("s p o -> p s o"))
            nc.sync.dma_start(fcb_s[:], fcb[:])
            for s in range(NCORES):
                nc.sync.dma_start(av_rx[:, s, :], a2a_out[s])

            for o4 in range(4):
                for tb0, tbw in FC_TB:
                    fps = fops.tile([128, 440], F32)
                    for s in range(NCORES):
                        nc.tensor.matmul(
                            fps[:, :tbw],
                            fcw_s[:, s, o4 * 128:(o4 + 1) * 128],
                            av_rx[:, s, tb0:tb0 + tbw],
                            start=(s == 0), stop=(s == NCORES - 1))
                    ot_t = fout.tile([128, 440], F32)
                    nc.vector.tensor_scalar(ot_t[:, :tbw], fps[:, :tbw],
                                            fcb_s[:, o4:o4 + 1], None, op0=ADD)
                    nc.sync.dma_start(out[o4 * 128:(o4 + 1) * 128,
                                          tb0:tb0 + tbw], ot_t[:, :tbw])


_NC_CACHE = None


def _get_nc():
    global _NC_CACHE
    if _NC_CACHE is None:
        _NC_CACHE = build_nc()
    return _NC_CACHE


def _prep_core_inputs(x, wq, wk, wv, fc_w, fc_b, base_mask, ot_mask, oo_mask):
    """Build the 8 per-core input maps (host-side shard/pack)."""
    x = np.asarray(x, np.float32)
    maps = []

    # per-batch packed token stream: [base(2048) | obj(4968)] transposed
    xT_b = []
    for b in range(B):
        gl = x[b, L_BBOX + NB:L_BBOX + NB + L_GLIM].reshape(T1, K, GG, DIM)
        bb = x[b, :L_BBOX].reshape(T2, K, 5, DIM)
        bbp = np.zeros((T1, K, 5, DIM), np.float32)
        bbp[:T2] = bb
        x_obj = np.concatenate([gl, bbp], axis=2).reshape(NO, DIM)
        x_base = x[b, L_BBOX:L_BBOX + NB]
        xa = np.concatenate([x_base, x_obj], axis=0)          # (7016, 512)
        xT_b.append(np.ascontiguousarray(xa.T.astype(np.float16)))

    def pack_w(w, cols):
        # (512, 128) head-pair columns -> (4, 128, 128) d-major tiles
        return np.ascontiguousarray(
            np.asarray(w, np.float32)[:, cols].reshape(4, 128, 128)
            .astype(np.float16))

    # masks (shared across cores): transposed to (k, q), padded, tiled
    def tile_mask(mT, nk):                                    # mT (k, q)
        kp = len(ktiles(nk)) * 128
        mp = np.zeros((kp, mT.shape[1]), np.uint8)
        mp[:nk] = mT.astype(np.uint8)
        return np.ascontiguousarray(mp.reshape(-1, 128, mT.shape[1]))

    mbb = tile_mask(np.asarray(base_mask).T, NB)              # (16,128,2048)
    mot = tile_mask(np.asarray(ot_mask).T, OT)                # (5,128,621)
    moo = np.stack([tile_mask(np.asarray(oo_mask[f]).T, FR) for f in range(T1)])

    fcb_t = np.ascontiguousarray(
        np.asarray(fc_b, np.float32).reshape(4, 128).T)       # (128, 4)

    fc_w = np.asarray(fc_w, np.float32)
    for c in range(NCORES):
        b, hp = c // 4, c % 4
        cols = slice(hp * 128, hp * 128 + 128)
        fcw_full = np.zeros((NCORES, 128, DIM), np.float16)
        grp = range(4) if b == 0 else range(4, 8)
        for s in grp:
            fcw_full[s] = fc_w[(s % 4) * 128:(s % 4) * 128 + 128, :].astype(
                np.float16)
        maps.append({
            "xT": xT_b[b],
            "wq": pack_w(wq, cols), "wk": pack_w(wk, cols),
            "wv": pack_w(wv, cols),
            "fcw": fcw_full, "fcb": fcb_t,
            "m_bb": mbb, "m_ot": mot, "m_oo": moo,
        })
    return maps


def _assemble(results):
    p_full = np.empty((B, NT, DIM), np.float32)
    for c in range(NCORES):
        b, sl = c // 4, c % 4
        p_full[b, sl * TS:(sl + 1) * TS] = results[c]["out"].T
    p_base = p_full[:, :NB].reshape(B, T2, 16, 16, DIM)
    obj = p_full[:, NB:].reshape(B, T1, K, GGP5, DIM)
    p_glimpses = np.ascontiguousarray(
        obj[..., :GG, :].reshape(B, T1, K, 8, 8, DIM))
    p_bboxes = np.ascontiguousarray(obj[:, :T2, :, GG:, :])
    return (p_bboxes, np.ascontiguousarray(p_base), p_glimpses)


def _ensure_ntff_hook():
    """Register the axon NTFF profile hook if the image's antenv lacks it."""
    import types
    try:
        from antenv.axon_hooks import get_axon_ntff_profile_hook  # noqa: F401
        return
    except ImportError:
        pass
    try:
        import antenv
        mod = types.ModuleType("antenv.axon_hooks")
        state = {"hook": None}
        mod.set_axon_ntff_profile_hook = lambda h: state.__setitem__("hook", h)
        mod.get_axon_ntff_profile_hook = lambda: state["hook"]
        sys.modules["antenv.axon_hooks"] = mod
        antenv.axon_hooks = mod
        if "/root/.axon_site" not in sys.path:
            sys.path.insert(0, "/root/.axon_site")
        from trn_agent_boot.trn_boot import _ntff_profile_via_ctypes
        hook = _ntff_profile_via_ctypes("/opt/axon/libaxon_pjrt.so")
        if hook is not None:
            mod.set_axon_ntff_profile_hook(hook)
    except Exception:
        pass


def run(trace=False, **inputs):
    if trace:
        _ensure_ntff_hook()
    nc = _get_nc()
    maps = _prep_core_inputs(**inputs)
    res = run_bass_kernel_spmd(nc, maps, list(range(NCORES)), trace=trace)
    return _assemble(res.results), res


def kernel(**inputs):
    out, _ = run(trace=False, **inputs)
    return out
